# revision 1
# baseline (speedup 1.0000x reference)
"""Two-layer GAT (PyG GATConv math) on 8 Trainium2 NeuronCores via Bass/Tile.

Sharding: nodes split into 8 contiguous ranges of 12500 (graph partitioning per
the sharding hint); each core aggregates the in-edges of its own nodes.
Within a core, nodes are relabeled in descending in-degree order so 128-node
windows have near-uniform degree (slot padding ~1.3%).

Per layer:
  node phase  - h = x @ W (PE matmuls) and attention scores es/ed (DVE),
                written as 512-B table rows [h(64 f32) | es(8) | pad];
                AllGather replicates the 100352-row table to every core.
  edge phase  - degree-padded dense layout: window w = 128 dst nodes on
                partitions, slot column k = k-th in-edge. Each column is
                fetched with one [128,1]-indexed indirect DMA (the only
                working data-dependent gather on this runtime). Softmax
                (max-subtraction elided: scores are O(5), exp safe in f32),
                weighted sum, then bias/activation - all nodes-on-partitions
                DVE/ACT ops. Pad slots point at a phantom row with es=-1e30
                so their exp weight is exactly 0.
"""
import sys

sys.path.insert(0, "/opt/trn_rl_repo")

import numpy as np

import concourse.bass as bass
import concourse.bacc as bacc
import concourse.tile as tile
from concourse import mybir
from concourse.bass import AP, IndirectOffsetOnAxis
from concourse.masks import make_identity

F32 = mybir.dt.float32
I32 = mybir.dt.int32
AX = mybir.AxisListType.X
OP = mybir.AluOpType
AF = mybir.ActivationFunctionType

N = 100_000
F_IN = 512
H1, FH1 = 8, 8
D1 = H1 * FH1          # 64
C = 64
NCORES = 8
NLR = N // NCORES      # 12500 real nodes per core
PW = 128
NWIN = (NLR + PW - 1) // PW   # 98
NL = NWIN * PW         # 12544 (44 phantom rows per core)
GT = NCORES * NL       # 100352 table rows
RL = 128               # table row stride in f32 (512 B)
NEG = -1.0e30
XB = 2                 # windows per x-load batch   (NWIN % XB == 0)
SB = 7                 # windows per table-staging batch (NWIN % SB == 0)


# ---------------------------------------------------------------- host planning
def _plan(edge_index):
    src = np.concatenate([edge_index[0], np.arange(N)]).astype(np.int64)
    dst = np.concatenate([edge_index[1], np.arange(N)]).astype(np.int64)
    owner = dst // NLR

    orders, posmaps, per_core = [], [], []
    for c in range(NCORES):
        m = owner == c
        s_c, d_c = src[m], dst[m] - c * NLR
        per_core.append((s_c, d_c))
        deg = np.bincount(d_c, minlength=NLR)
        order = np.argsort(-deg, kind="stable")
        posmap = np.empty(NLR, dtype=np.int64)
        posmap[order] = np.arange(NLR)
        orders.append(order)
        posmaps.append(posmap)

    K = np.ones(NWIN, dtype=np.int64)
    for c in range(NCORES):
        deg = np.bincount(per_core[c][1], minlength=NLR)
        dpad = np.zeros(NL, dtype=np.int64)
        dpad[:NLR] = deg[orders[c]]
        K = np.maximum(K, dpad.reshape(NWIN, PW).max(axis=1))
    offs = np.concatenate([[0], np.cumsum(K)]).astype(np.int64)
    tot = int(offs[-1])

    idx_streams = []
    for c in range(NCORES):
        s_c, d_c = per_core[c]
        pos = posmaps[c][d_c]
        srow = np.empty(len(s_c), dtype=np.int64)
        so = s_c // NLR
        for o in range(NCORES):
            mo = so == o
            srow[mo] = o * NL + posmaps[o][s_c[mo] - o * NLR]
        ordd = np.argsort(pos, kind="stable")
        pos_s, srow_s = pos[ordd], srow[ordd]
        cnt = np.bincount(pos_s, minlength=NL)
        starts = np.concatenate([[0], np.cumsum(cnt)])[:-1]
        k_of = np.arange(len(pos_s)) - starts[pos_s]
        pad_row = c * NL + NL - 1
        idxmat = np.full((128, tot), pad_row, dtype=np.int32)
        idxmat[pos_s & 127, offs[pos_s >> 7] + k_of] = srow_s
        idx_streams.append(idxmat)

    return {"orders": orders, "K": K, "offs": offs, "tot": tot, "idx": idx_streams}


def _apx(base: AP, off: int, dims) -> AP:
    """AP with base's partition dim and explicit free [step, count] dims."""
    return AP(base.tensor, base.offset + off, [list(base.ap[0])] + [list(d) for d in dims])


# ---------------------------------------------------------------- device build
def _build(K, tot, offs):
    K = [int(v) for v in K]
    offs = [int(v) for v in offs]
    K0 = max(K)

    nc = bacc.Bacc("TRN2", target_bir_lowering=False, debug=False, num_devices=NCORES)

    xT = nc.dram_tensor("xT", [F_IN, NL], F32, kind="ExternalInput")
    w1 = nc.dram_tensor("w1", [F_IN, D1], F32, kind="ExternalInput")
    w2 = nc.dram_tensor("w2", [D1, C], F32, kind="ExternalInput")
    cvec = nc.dram_tensor("cvec", [128, 6 * 64], F32, kind="ExternalInput")
    negd = nc.dram_tensor("negd", [NL - NLR, RL], F32, kind="ExternalInput")
    idxd = nc.dram_tensor("idxd", [128, tot], I32, kind="ExternalInput")
    outd = nc.dram_tensor("outv", [NL, C], F32, kind="ExternalOutput")

    t1b = nc.dram_tensor("t1b", [NL, RL], F32)
    T1 = nc.dram_tensor("T1", [GT, RL], F32, addr_space="Shared")
    t2b = nc.dram_tensor("t2b", [NL, RL], F32)
    T2 = nc.dram_tensor("T2", [GT, RL], F32, addr_space="Shared")

    with tile.TileContext(nc) as tc:
        with (
            tc.tile_pool(name="consts", bufs=1) as cpool,
            tc.tile_pool(name="persist", bufs=1) as ppool,
            tc.tile_pool(name="xload", bufs=2) as xpool,
            tc.tile_pool(name="stg", bufs=2) as stgpool,
            tc.tile_pool(name="gpool", bufs=2) as gpool,
            tc.tile_pool(name="zpool", bufs=2) as zpool,
            tc.tile_pool(name="small", bufs=2) as spool,
            tc.tile_pool(name="psum", bufs=4, space="PSUM") as pspool,
        ):
            # ---- constants (packed)
            w1sb = cpool.tile([128, 4 * D1], F32)
            nc.sync.dma_start(out=w1sb[:].rearrange("p (cc d) -> p cc d", cc=4), in_=w1[:, :].rearrange("(cc p) d -> p cc d", p=128))
            w2sb = cpool.tile([128, C], F32)
            nc.sync.dma_start(out=w2sb[:D1, :], in_=w2[:, :])
            cv = cpool.tile([128, 6 * 64], F32)
            nc.sync.dma_start(out=cv[:], in_=cvec[:, :])
            asrs = cv[:, 0:64]
            adss = cv[:, 64:128]
            a2ss = cv[:, 128:192]
            a2ds = cv[:, 192:256]
            b1s = cv[:, 256:320]
            b2s = cv[:, 320:384]
            ident = cpool.tile([128, 128], F32)
            make_identity(nc, ident[:])
            idxs = cpool.tile([128, tot], I32)
            nc.sync.dma_start(out=idxs[:], in_=idxd[:, :])

            # ---- persistent
            x2st = ppool.tile([128, NWIN * D1], F32)
            edt = ppool.tile([128, NWIN * H1 + NWIN], F32)  # ed1 | ed2

            def node_phase(layer):
                tb, Tg = (t1b, T1) if layer == 1 else (t2b, T2)
                for sb in range(0, NWIN, SB):
                    stg = stgpool.tile([128, SB * RL], F32, tag="stg")
                    nc.vector.memset(stg[:], 0.0)
                    for w in range(sb, sb + SB):
                        wl = w - sb
                        if layer == 1 and w % XB == 0:
                            xb = xpool.tile([128, 4 * XB * 128], F32, tag="xb")
                            nc.sync.dma_start(
                                out=xb[:].rearrange("p (cc n) -> p cc n", cc=4),
                                in_=xT[:, w * 128 : (w + XB) * 128].rearrange(
                                    "(cc p) n -> p cc n", p=128
                                ),
                            )
                        ph = pspool.tile([128, D1], F32, tag="ph")
                        if layer == 1:
                            nn = XB * 128
                            for cc in range(4):
                                nc.tensor.matmul(
                                    out=ph[:],
                                    lhsT=_apx(xb[:], cc * nn + (w % XB) * 128, [[1, 128]]),
                                    rhs=_apx(w1sb[:], cc * D1, [[1, D1]]),
                                    start=(cc == 0),
                                    stop=(cc == 3),
                                )
                        else:
                            pt = pspool.tile([64, 128], F32, tag="pt")
                            nc.tensor.transpose(
                                out=pt[:],
                                in_=_apx(x2st[:], w * D1, [[1, D1]]),
                                identity=ident[:],
                            )
                            x1t = spool.tile([64, 128], F32, tag="x1t")
                            nc.vector.tensor_copy(out=x1t[:], in_=pt[:])
                            nc.tensor.matmul(
                                out=ph[:], lhsT=x1t[:], rhs=w2sb[:D1, :],
                                start=True, stop=True,
                            )
                        hcol = _apx(stg[:], wl * RL, [[1, D1]])
                        nc.vector.tensor_copy(out=hcol, in_=ph[:])
                        a_s = asrs if layer == 1 else a2ss
                        a_d = adss if layer == 1 else a2ds
                        tmp = spool.tile([128, 2 * D1], F32, tag="tmp")
                        nc.vector.tensor_tensor(out=tmp[:, :D1], in0=hcol, in1=a_s, op=OP.mult)
                        nc.vector.tensor_tensor(out=tmp[:, D1:], in0=hcol, in1=a_d, op=OP.mult)
                        if layer == 1:
                            nc.vector.tensor_reduce(
                                out=_apx(stg[:], wl * RL + D1, [[1, H1]]),
                                in_=_apx(tmp[:], 0, [[FH1, H1], [1, FH1]]),
                                axis=AX, op=OP.add)
                            nc.vector.tensor_reduce(
                                out=_apx(edt[:], w * H1, [[1, H1]]),
                                in_=_apx(tmp[:], D1, [[FH1, H1], [1, FH1]]),
                                axis=AX, op=OP.add)
                        else:
                            nc.vector.tensor_reduce(
                                out=_apx(stg[:], wl * RL + D1, [[1, 1]]),
                                in_=_apx(tmp[:], 0, [[1, C]]),
                                axis=AX, op=OP.add)
                            nc.vector.tensor_reduce(
                                out=_apx(edt[:], NWIN * H1 + w, [[1, 1]]),
                                in_=_apx(tmp[:], D1, [[1, C]]),
                                axis=AX, op=OP.add)
                    nc.sync.dma_start(
                        out=tb[sb * 128 : (sb + SB) * 128, :].rearrange(
                            "(w p) r -> p w r", p=128
                        ),
                        in_=stg[:].rearrange("p (w r) -> p w r", w=SB),
                    )
                # phantom rows (the padding-slot target) -> giant negative es
                nc.sync.dma_start(out=tb[NLR:NL, :], in_=negd[:, :])
                nc.gpsimd.collective_compute(
                    "AllGather", OP.bypass,
                    replica_groups=[list(range(NCORES))],
                    ins=[tb[:, :]], outs=[Tg[:, :]],
                )

            def edge_phase(layer):
                Tg = T1 if layer == 1 else T2
                for w in range(NWIN):
                    Kw = K[w]
                    G = gpool.tile([128, K0 * RL], F32, tag="G")
                    for k in range(Kw):
                        nc.gpsimd.indirect_dma_start(
                            out=_apx(G[:], k * RL, [[1, RL]]),
                            out_offset=None,
                            in_=Tg[:, :],
                            in_offset=IndirectOffsetOnAxis(
                                ap=idxs[:, offs[w] + k : offs[w] + k + 1], axis=0
                            ),
                        )
                    dn = spool.tile([128, 128], F32, tag="dn")
                    if layer == 1:
                        z = zpool.tile([128, H1 * K0], F32, tag="z")
                        nc.vector.tensor_tensor(
                            out=_apx(z[:], 0, [[Kw, H1], [1, Kw]]),
                            in0=_apx(G[:], D1, [[1, H1], [RL, Kw]]),
                            in1=_apx(edt[:], w * H1, [[1, H1], [0, Kw]]),
                            op=OP.add)
                        zf = _apx(z[:], 0, [[1, H1 * Kw]])
                        nc.vector.scalar_tensor_tensor(
                            out=zf, in0=zf, scalar=0.2, in1=zf, op0=OP.mult, op1=OP.max)
                        nc.scalar.activation(out=zf, in_=zf, func=AF.Exp)
                        nc.vector.tensor_reduce(
                            out=dn[:, 0:H1],
                            in_=_apx(z[:], 0, [[Kw, H1], [1, Kw]]),
                            axis=AX, op=OP.add)
                        gh = _apx(G[:], 0, [[RL, Kw], [FH1, H1], [1, FH1]])
                        nc.vector.tensor_tensor(
                            out=gh, in0=gh,
                            in1=_apx(z[:], 0, [[1, Kw], [Kw, H1], [0, FH1]]),
                            op=OP.mult)
                        nc.vector.tensor_reduce(
                            out=dn[:, 64:128],
                            in_=_apx(G[:], 0, [[FH1, H1], [1, FH1], [RL, Kw]]),
                            axis=AX, op=OP.add)
                        nc.vector.reciprocal(out=dn[:, 8:16], in_=dn[:, 0:H1])
                        nc.vector.tensor_tensor(
                            out=_apx(x2st[:], w * D1, [[1, D1]]),
                            in0=dn[:, 64:128],
                            in1=_apx(dn[:], 8, [[1, H1], [0, FH1]]),
                            op=OP.mult)
                    else:
                        z = zpool.tile([128, H1 * K0], F32, tag="z")
                        nc.vector.tensor_tensor(
                            out=_apx(z[:], 0, [[1, Kw]]),
                            in0=_apx(G[:], D1, [[RL, Kw]]),
                            in1=_apx(edt[:], NWIN * H1 + w, [[0, Kw]]),
                            op=OP.add)
                        zf = _apx(z[:], 0, [[1, Kw]])
                        nc.vector.scalar_tensor_tensor(
                            out=zf, in0=zf, scalar=0.2, in1=zf, op0=OP.mult, op1=OP.max)
                        nc.scalar.activation(out=zf, in_=zf, func=AF.Exp)
                        nc.vector.tensor_reduce(out=dn[:, 0:1], in_=zf, axis=AX, op=OP.add)
                        # phantom lanes: den=0 -> 0/0 NaN poisons final Ln; make it 0/eps=0
                        nc.vector.tensor_scalar_add(dn[:, 0:1], dn[:, 0:1], 1e-30)
                        gh = _apx(G[:], 0, [[RL, Kw], [1, C]])
                        nc.vector.tensor_tensor(
                            out=gh, in0=gh,
                            in1=_apx(z[:], 0, [[1, Kw], [0, C]]), op=OP.mult)
                        nc.vector.tensor_reduce(
                            out=dn[:, 64:128],
                            in_=_apx(G[:], 0, [[1, C], [RL, Kw]]),
                            axis=AX, op=OP.add)
                        nc.vector.reciprocal(out=dn[:, 1:2], in_=dn[:, 0:1])
                        nc.vector.tensor_tensor(
                            out=_apx(x2st[:], w * C, [[1, C]]),
                            in0=dn[:, 64:128],
                            in1=_apx(dn[:], 1, [[0, C]]),
                            op=OP.mult)

            # ================= layer 1 =================
            node_phase(1)
            edge_phase(1)
            # x1 = elu(x2st + b1), chunked
            for g in range(0, NWIN, SB):
                xs = _apx(x2st[:], g * D1, [[1, SB * D1]])
                nc.vector.tensor_tensor(
                    out=xs, in0=xs, in1=_apx(b1s, 0, [[0, SB], [1, D1]]), op=OP.add)
                tmp = spool.tile([128, SB * D1], F32, tag="tail")
                tf = _apx(tmp[:], 0, [[1, SB * D1]])
                nc.vector.tensor_scalar_min(tf, xs, 0.0)
                nc.scalar.activation(out=tf, in_=tf, func=AF.Exp)
                nc.vector.tensor_scalar_max(xs, xs, 0.0)
                nc.vector.scalar_tensor_tensor(
                    out=xs, in0=tf, scalar=-1.0, in1=xs, op0=OP.add, op1=OP.add)

            # ================= layer 2 =================
            node_phase(2)
            edge_phase(2)
            # out = log_softmax(x2st + b2), chunked
            for g in range(0, NWIN, SB):
                xs = _apx(x2st[:], g * C, [[1, SB * C]])
                nc.vector.tensor_tensor(
                    out=xs, in0=xs, in1=_apx(b2s, 0, [[0, SB], [1, C]]), op=OP.add)
                rmx = spool.tile([128, SB], F32, tag="rmx")
                nc.vector.tensor_reduce(
                    out=rmx[:], in_=_apx(x2st[:], g * C, [[C, SB], [1, C]]),
                    axis=AX, op=OP.max)
                nc.vector.tensor_tensor(
                    out=xs, in0=xs, in1=_apx(rmx[:], 0, [[1, SB], [0, C]]),
                    op=OP.subtract)
                tmp = spool.tile([128, SB * C], F32, tag="tail")
                tf = _apx(tmp[:], 0, [[1, SB * C]])
                nc.scalar.activation(out=tf, in_=xs, func=AF.Exp)
                nc.vector.tensor_reduce(
                    out=rmx[:], in_=_apx(tmp[:], 0, [[C, SB], [1, C]]),
                    axis=AX, op=OP.add)
                nc.scalar.activation(out=rmx[:], in_=rmx[:], func=AF.Ln)
                nc.vector.tensor_tensor(
                    out=xs, in0=xs, in1=_apx(rmx[:], 0, [[1, SB], [0, C]]),
                    op=OP.subtract)
            nc.sync.dma_start(
                out=outd[:, :].rearrange("(w p) f -> p w f", p=128),
                in_=x2st[:].rearrange("p (w f) -> p w f", w=NWIN),
            )

    nc.compile()
    return nc


# ---------------------------------------------------------------- PJRT runner
def _make_runner(nc):
    import jax
    from jax.sharding import Mesh, PartitionSpec, NamedSharding
    from jax.experimental.shard_map import shard_map
    from concourse import bass2jax
    from concourse.bass2jax import _bass_exec_p, install_neuronx_cc_hook

    install_neuronx_cc_hook()
    partition_name = nc.partition_id_tensor.name if nc.partition_id_tensor else None
    in_names, out_names, out_avals = [], [], []
    for alloc in nc.m.functions[0].allocations:
        if not isinstance(alloc, mybir.MemoryLocationSet):
            continue
        name = alloc.memorylocations[0].name
        if alloc.kind == "ExternalInput":
            if name != partition_name:
                in_names.append(name)
        elif alloc.kind == "ExternalOutput":
            out_avals.append(
                jax.core.ShapedArray(tuple(alloc.tensor_shape), mybir.dt.np(alloc.dtype))
            )
            out_names.append(name)
    n_params = len(in_names)
    all_in = list(in_names) + list(out_names)
    if partition_name is not None:
        all_in.append(partition_name)

    def _body(*args):
        operands = list(args)
        if partition_name is not None:
            operands.append(bass2jax.partition_id_tensor())
        return tuple(
            _bass_exec_p.bind(
                *operands,
                out_avals=tuple(out_avals),
                in_names=tuple(all_in),
                out_names=tuple(out_names),
                lowering_input_output_aliases=(),
                sim_require_finite=True,
                sim_require_nnan=True,
                nc=nc,
            )
        )

    devices = jax.devices()[:NCORES]
    mesh = Mesh(np.asarray(devices), ("core",))
    n_outs = len(out_names)
    sharded = jax.jit(
        shard_map(
            _body, mesh=mesh,
            in_specs=(PartitionSpec("core"),) * (n_params + n_outs),
            out_specs=(PartitionSpec("core"),) * n_outs,
            check_rep=False,
        ),
        keep_unused=True,
    )
    sharding = NamedSharding(mesh, PartitionSpec("core"))

    def run(in_maps):
        import jax as _jax

        per_core = [[np.asarray(m[nm]) for nm in in_names] for m in in_maps]
        concat_in = [
            np.concatenate([per_core[c][i] for c in range(NCORES)], axis=0)
            for i in range(n_params)
        ]
        concat_zero = [
            np.zeros((NCORES * a.shape[0], *a.shape[1:]), a.dtype) for a in out_avals
        ]
        args = [_jax.device_put(x, sharding) for x in concat_in + concat_zero]
        out = sharded(*args)
        _jax.block_until_ready(out)
        return (
            [
                {
                    nm: np.asarray(out[i]).reshape(NCORES, *out_avals[i].shape)[c]
                    for i, nm in enumerate(out_names)
                }
                for c in range(NCORES)
            ],
            sharded,
            args,
        )

    return run


_CACHE = {}


def _get_compiled(K, tot, offs):
    key = (tot, tuple(int(v) for v in K))
    if key not in _CACHE:
        nc = _build(K, tot, offs)
        _CACHE[key] = (nc, _make_runner(nc))
    return _CACHE[key]


def _prep_inputs(x, plan, W1, att1_src, att1_dst, b1, W2, att2_src, att2_dst, b2):
    cvec = np.zeros((128, 6 * 64), np.float32)
    cvec[:, 0:64] = att1_src.reshape(1, D1)
    cvec[:, 64:128] = att1_dst.reshape(1, D1)
    cvec[:, 128:192] = att2_src.reshape(1, C)
    cvec[:, 192:256] = att2_dst.reshape(1, C)
    cvec[:, 256:320] = b1.reshape(1, D1)
    cvec[:, 320:384] = b2.reshape(1, C)
    in_maps = []
    for c in range(NCORES):
        order = plan["orders"][c]
        xp = np.zeros((NL, F_IN), np.float32)
        xp[:NLR] = x[c * NLR : (c + 1) * NLR][order]
        in_maps.append(
            {
                "xT": np.ascontiguousarray(xp.T),
                "w1": np.ascontiguousarray(np.asarray(W1, np.float32)),
                "w2": np.ascontiguousarray(np.asarray(W2, np.float32)),
                "cvec": cvec,
                "negd": np.full((NL - NLR, RL), NEG, np.float32),
                "idxd": plan["idx"][c],
            }
        )
    return in_maps


def kernel(x, edge_index, W1, att1_src, att1_dst, b1, W2, att2_src, att2_dst, b2):
    x = np.asarray(x, np.float32)
    edge_index = np.asarray(edge_index)
    plan = _plan(edge_index)
    nc, run = _get_compiled(plan["K"], plan["tot"], plan["offs"])
    in_maps = _prep_inputs(
        x, plan,
        np.asarray(W1), np.asarray(att1_src), np.asarray(att1_dst), np.asarray(b1),
        np.asarray(W2), np.asarray(att2_src), np.asarray(att2_dst), np.asarray(b2),
    )
    results, _, _ = run(in_maps)
    out = np.empty((N, C), np.float32)
    for c in range(NCORES):
        out[c * NLR + plan["orders"][c]] = results[c]["outv"][:NLR]
    return out

